# revision 9
# baseline (speedup 1.0000x reference)
"""Causal linear attention (ELU+1 feature map) on 8 trn2 NeuronCores.

Sharding: core i handles batch b=i//2, sequence half h=i%2 (T=2048 -> 1024
tokens/core).  Second-half cores recompute the first half's running state
g_pre = sum_tau phi(k_tau) [v_tau, 1]  (128x129, col 128 = z) from k/v of the
first half; first-half cores get zeroed aux inputs so their g_pre == 0.

Key identity: phi(y) = ELU(y)+1 = min(exp(y), max(y+1, 1))
  -> per 512-col block: ACT exp, one tensor_scalar (add bias+1, max 1),
     one tensor_tensor min.

State handled as independent per-chunk-pair tiles g[j] (no serial snapshot
chain):  O_c = Am_c^T.T @ [V_c,1] + Q_c @ g_pre + sum_{j<c} Q_c @ g_j
  out_c = O_c[:, :128] * (1 / O_c[:, 128])

Token-major phi(K) for the state matmuls comes from DMA-engine transposes
(dma_start_transpose) of the feature-major phi(K), or PE transposes (cfg).
"""

import numpy as np

B, T, D, DV = 4, 2048, 128, 128
H = T // 2          # tokens per core
C = 128             # chunk
NCH = H // C        # chunks per half (8)
NCORES = 8
VW = DV + 1

# bf16 pack columns
OFF_WTB = 0
OFF_MASK4 = OFF_WTB + D          # 512-wide four-chunk causal mask
OFF_ID = OFF_MASK4 + 4 * C       # identity (PE transpose)
OFF_B = OFF_ID + C               # bias col
OFF_B1 = OFF_B + 1               # bias+1 col
OFF_KT = OFF_B1 + 1
OFF_KTP = OFF_KT + H
OFF_QT = OFF_KTP + H
OFF_VP = OFF_QT + H
OFF_V = OFF_VP + NCH * VW
B16_COLS = OFF_V + NCH * VW

CFG = {
    # DMA piece order: list of (colstart, colend, queue) — queue: sp|dve|act
    "pieces": [
        (OFF_WTB, OFF_KT, "sp"),          # consts
        (OFF_KT, OFF_KT + H, "sp"),       # kT
        (OFF_KTP, OFF_KTP + H, "act"),    # kTp
        (OFF_VP, OFF_VP + NCH * VW, "pool"),  # vp
        (OFF_QT, OFF_QT + H, "sp"),       # qT
        (OFF_V, B16_COLS, "sp"),          # v
    ],
    # engines per phi block [K0, K1, Q0, Q1, P0, P1]
    "blk_style": ["fused", "fused", "fused", "fused", "reluact", "reluact"],
    "tt_eng": ["dve", "dve", "dve", "dve", "dve", "dve"],
    "ktok_mode": "pe",       # dmat | pe
    "ktok_copy": ["act", "dve"],   # per 4-chunk transpose batch
    "t_queue": "sync",
    "mask_eng": ["dve", "dve"],   # per quad
    "gcopy_eng": ["dve", "act", "dve", "act", "act"],  # 4 pairs + pre
    "scale_eng": ["dve", "act", "dve", "act", "dve", "act", "dve", "act"],
    "o_first": "av",          # first matmul in each O chain
    "out_pieces": [(0, 512, "sp"), (512, 896, "sp"), (896, 1024, "sp")],
    "o_bufs": 2,
    "emit": "default",
}

_cache = {}


def _build(cfg=None):
    import concourse.bacc as bacc
    import concourse.tile as tile
    from concourse import mybir
    from bass_rust import add_dep_helper

    cfg = dict(CFG, **(cfg or {}))
    F32 = mybir.dt.float32
    BF16 = mybir.dt.bfloat16
    AF = mybir.ActivationFunctionType
    ALU = mybir.AluOpType

    nc = bacc.Bacc(None, target_bir_lowering=False, debug=False,
                   num_devices=NCORES)

    bin_ = nc.declare_dram_parameter("bin", [D, B16_COLS], BF16, isOutput=False)
    btile = nc.declare_dram_parameter("btile", [1, 512], BF16, isOutput=False)
    out = nc.declare_dram_parameter("out", [C, NCH * DV], BF16, isOutput=True)

    def dma_eng(which):
        return {"sp": nc.sync, "dve": nc.vector, "act": nc.scalar,
                "pool": nc.gpsimd}[which]

    with tile.TileContext(nc) as tc:
        with (
            tc.tile_pool(name="cst", bufs=1) as cst,
            tc.tile_pool(name="io", bufs=1) as io,
            tc.tile_pool(name="phi", bufs=1) as phip,
            tc.tile_pool(name="wrk", bufs=2) as wrk,
            tc.tile_pool(name="ps_phi", bufs=2, space="PSUM") as ps_phi,
            tc.tile_pool(name="ps_tr", bufs=1, space="PSUM") as ps_tr,
            tc.tile_pool(name="ps_ag", bufs=3, space="PSUM") as ps_ag,
            tc.tile_pool(name="ps_o", bufs=cfg["o_bufs"], space="PSUM") as ps_o,
        ):
            # ---- warm the ACT table while DMAs run ----
            s_warm = cst.tile([D, 1], F32, name="s_warm")
            nc.vector.memset(s_warm, 0.0)
            s_warm2 = cst.tile([D, 1], BF16, name="s_warm2")
            nc.scalar.activation(s_warm2, s_warm, AF.Exp)

            # ---- loads ----
            s_b16 = io.tile([D, B16_COLS], BF16, name="s_b16")
            s_btile = cst.tile([1, 512], BF16, name="s_btile")
            s_ones = cst.tile([1, C], BF16, name="s_ones")
            nc.sync.dma_start(out=s_btile, in_=btile[:, :])
            nc.vector.memset(s_ones, 1.0)
            for (a, b, q) in cfg["pieces"]:
                dma_eng(q).dma_start(out=s_b16[:, a:b], in_=bin_[:, a:b])

            s_b = s_b16[:, OFF_B:OFF_B + 1]
            s_b1_32 = cst.tile([D, 1], F32, name="s_b1_32")
            nc.vector.tensor_copy(s_b1_32, s_b16[:, OFF_B1:OFF_B1 + 1])
            sWTb = s_b16[:, OFF_WTB:OFF_WTB + D]
            s_mask4 = s_b16[:, OFF_MASK4:OFF_MASK4 + 4 * C]

            def vsl(c):
                return s_b16[:, OFF_V + VW * c:OFF_V + VW * (c + 1)]

            def vpsl(c):
                return s_b16[:, OFF_VP + VW * c:OFF_VP + VW * (c + 1)]

            phiK = phip.tile([D, H], BF16, name="phiK")
            phiQ = phip.tile([D, H], BF16, name="phiQ")
            phiT = phip.tile([C, H], BF16, name="phiT")   # token-major pre
            ktok = phip.tile([C, H], BF16, name="ktok")
            outstage = phip.tile([C, NCH * DV], BF16, name="outstage")

            # ---- feature-major phi block: dst[:, j*512:(j+1)*512] ----
            def phi_feat(dst, src_off, j, bi):
                pre = ps_phi.tile([D, 512], F32, tag="pp", name="pre")
                nc.tensor.matmul(pre, sWTb,
                                 s_b16[:, src_off + 512 * j:src_off + 512 * (j + 1)],
                                 start=True, stop=True)
                sl = slice(512 * j, 512 * (j + 1))
                e_t = wrk.tile([D, 512], BF16, tag="e", name="e_t")
                r_t = wrk.tile([D, 512], BF16, tag="r", name="r_t")
                nc.scalar.activation(e_t, pre, AF.Exp, bias=s_b, scale=1.0)
                if cfg["blk_style"][bi] == "fused":
                    # r1 = max(y+b+1, 1) on DVE from PSUM
                    nc.vector.tensor_scalar(out=r_t, in0=pre, scalar1=s_b1_32,
                                            scalar2=1.0, op0=ALU.add,
                                            op1=ALU.max)
                else:
                    # r = relu(y+b) on ACT, then r1 = r+1 on DVE (4x)
                    r0 = wrk.tile([D, 512], BF16, tag="r0", name="r0")
                    nc.scalar.activation(r0, pre, AF.Relu, bias=s_b, scale=1.0)
                    nc.vector.tensor_scalar(out=r_t, in0=r0, scalar1=1.0,
                                            scalar2=None, op0=ALU.add)
                nc.vector.tensor_tensor(out=dst[:, sl], in0=e_t, in1=r_t,
                                        op=ALU.min)

            # ---- token-major phi block (bias via ones-matmul) ----
            def phi_tok(dst, src_off, j, bi):
                pst = ps_phi.tile([C, 512], F32, tag="pp", name="pst")
                prev = nc.tensor.matmul(pst, s_ones, s_btile[:, :],
                                        start=True, stop=False)
                for cc in range(4):
                    mm = nc.tensor.matmul(
                        pst[:, C * cc:C * (cc + 1)],
                        s_b16[:, src_off + 512 * j + C * cc:
                              src_off + 512 * j + C * (cc + 1)],
                        sWTb, start=False, stop=(cc == 3))
                    add_dep_helper(mm.ins, prev.ins, sync=False,
                                   reason="psum group order")
                    prev = mm
                sl = slice(512 * j, 512 * (j + 1))
                e_t = wrk.tile([C, 512], BF16, tag="e", name="e_t")
                r_t = wrk.tile([C, 512], BF16, tag="r", name="r_t")
                nc.scalar.activation(e_t, pst, AF.Exp)
                if cfg["blk_style"][bi] == "fused":
                    nc.vector.tensor_scalar(out=r_t, in0=pst, scalar1=1.0,
                                            scalar2=1.0, op0=ALU.add,
                                            op1=ALU.max)
                else:
                    r0 = wrk.tile([C, 512], BF16, tag="r0", name="r0")
                    nc.scalar.activation(r0, pst, AF.Relu)
                    nc.vector.tensor_scalar(out=r_t, in0=r0, scalar1=1.0,
                                            scalar2=None, op0=ALU.add)
                nc.vector.tensor_tensor(out=dst[:, sl], in0=e_t, in1=r_t,
                                        op=ALU.min)

            # ---- ktok: token-major own-K ----
            def ktok_half(j):
                if cfg["ktok_mode"] == "dmat":
                    k3 = ktok[:, 512 * j:512 * (j + 1)].rearrange(
                        "p (c w) -> p c w", c=4)
                    dma_eng({"sync": "sp"}.get(cfg["t_queue"], cfg["t_queue"])
                            ).dma_start_transpose(k3, phiK[:, 512 * j:512 * (j + 1)])
                else:
                    trp = ps_tr.tile([C, 512], BF16, tag="tr", name="trp")
                    prev = None
                    for cc in range(4):
                        c = 4 * j + cc
                        mm = nc.tensor.transpose(trp[:, C * cc:C * (cc + 1)],
                                                 phiK[:, C * c:C * (c + 1)],
                                                 s_ident)
                        if prev is not None:
                            add_dep_helper(mm.ins, prev.ins, sync=False,
                                           reason="psum order")
                        prev = mm
                    eng = cfg["ktok_copy"][j]
                    sl = slice(512 * j, 512 * (j + 1))
                    if eng == "act":
                        nc.scalar.activation(ktok[:, sl], trp, AF.Copy)
                    elif eng == "pool":
                        nc.gpsimd.tensor_copy(ktok[:, sl], trp)
                    else:
                        nc.vector.tensor_copy(ktok[:, sl], trp)

            s_ident = s_b16[:, OFF_ID:OFF_ID + C]

            # ---- pre-state: 8 matmuls into one PSUM tile -> g_pre ----
            g_pre = phip.tile([D, VW], BF16, name="g_pre")

            def pre_state():
                S = ps_ag.tile([D, VW], F32, tag="ag", name="S")
                prev = None
                for c in range(NCH):
                    mm = nc.tensor.matmul(S, phiT[:, C * c:C * (c + 1)],
                                          vpsl(c), start=(c == 0),
                                          stop=(c == NCH - 1),
                                          skip_group_check=True)
                    if prev is not None:
                        add_dep_helper(mm.ins, prev.ins, sync=False,
                                       reason="psum group order")
                    prev = mm
                eng = cfg["gcopy_eng"][4]
                if eng == "act":
                    nc.scalar.activation(g_pre, S, AF.Copy)
                else:
                    nc.vector.tensor_copy(g_pre, S)

            # ---- G pairs ----
            g = [None] * 4

            def g_pair(j):
                Gp = ps_ag.tile([D, 2 * VW], F32, tag="ag", name="Gp")
                m0 = nc.tensor.matmul(Gp[:, 0:VW], ktok[:, C * 2 * j:C * (2 * j + 1)],
                                      vsl(2 * j), start=True, stop=True,
                                      skip_group_check=True)
                m1 = nc.tensor.matmul(Gp[:, VW:2 * VW],
                                      ktok[:, C * (2 * j + 1):C * (2 * j + 2)],
                                      vsl(2 * j + 1), start=True, stop=True,
                                      skip_group_check=True)
                add_dep_helper(m1.ins, m0.ins, sync=False, reason="psum order")
                gj = phip.tile([D, 2 * VW], BF16, name=f"g{j}")
                g[j] = gj
                eng = cfg["gcopy_eng"][j]
                if eng == "act":
                    nc.scalar.activation(gj, Gp, AF.Copy)
                else:
                    nc.vector.tensor_copy(gj, Gp)

            # ---- A quads + mask ----
            Am = [None] * 2

            def a_quad(j):
                Ap = ps_ag.tile([C, 4 * C], F32, tag="ag", name="Ap")
                prev = None
                for cc in range(4):
                    c = 4 * j + cc
                    mm = nc.tensor.matmul(Ap[:, C * cc:C * (cc + 1)],
                                          phiK[:, C * c:C * (c + 1)],
                                          phiQ[:, C * c:C * (c + 1)],
                                          start=True, stop=True,
                                          skip_group_check=True)
                    if prev is not None:
                        add_dep_helper(mm.ins, prev.ins, sync=False,
                                       reason="psum order")
                    prev = mm
                amj = phip.tile([C, 4 * C], BF16, name=f"am{j}")
                Am[j] = amj
                eng = cfg["mask_eng"][j]
                e = nc.vector if eng == "dve" else nc.gpsimd
                e.tensor_tensor(out=amj, in0=Ap, in1=s_mask4, op=ALU.mult)

            # ---- O chunks ----
            def o_chunk(c, Ot):
                half = c % 2
                osl = slice(half * VW, (half + 1) * VW)
                mms = []
                if cfg["o_first"] == "av":
                    mms.append(("av", None))
                    mms.append(("gpre", None))
                else:
                    mms.append(("gpre", None))
                    mms.append(("av", None))
                for j in range(c):
                    mms.append(("g", j))
                prev = None
                qsl = phiQ[:, C * c:C * (c + 1)]
                for i, (kind, j) in enumerate(mms):
                    start, stop = (i == 0), (i == len(mms) - 1)
                    if kind == "av":
                        amj = Am[c // 4]
                        mm = nc.tensor.matmul(
                            Ot[:, osl], amj[:, (c % 4) * C:(c % 4 + 1) * C],
                            vsl(c), start=start, stop=stop,
                            skip_group_check=True)
                    elif kind == "gpre":
                        mm = nc.tensor.matmul(Ot[:, osl], qsl, g_pre,
                                              start=start, stop=stop,
                                              skip_group_check=True)
                    else:
                        gj = g[j // 2]
                        gslice = gj[:, (j % 2) * VW:(j % 2 + 1) * VW]
                        mm = nc.tensor.matmul(Ot[:, osl], qsl, gslice,
                                              start=start, stop=stop,
                                              skip_group_check=True)
                    if prev is not None:
                        add_dep_helper(mm.ins, prev.ins, sync=False,
                                       reason="psum group order")
                    prev = mm
                # scale
                eng = cfg["scale_eng"][c]
                dsl = outstage[:, DV * c:DV * (c + 1)]
                ssl = Ot[:, half * VW:half * VW + DV]
                den = Ot[:, half * VW + DV:half * VW + DV + 1]
                if eng == "div":
                    nc.vector.tensor_scalar(out=dsl, in0=ssl, scalar1=den,
                                            scalar2=None, op0=ALU.divide)
                else:
                    rec = wrk.tile([C, 1], F32, tag="rec", name="rec")
                    nc.vector.reciprocal(rec, den)
                    if eng == "act":
                        nc.scalar.activation(dsl, ssl, AF.Copy, bias=0.0,
                                             scale=rec)
                    else:
                        nc.vector.tensor_scalar_mul(dsl, ssl, rec)

            # ================= emission =================
            phi_feat(phiK, OFF_KT, 0, 0)
            phi_feat(phiK, OFF_KT, 1, 1)
            ktok_half(0)
            ktok_half(1)
            phi_tok(phiT, OFF_KTP, 0, 4)
            phi_tok(phiT, OFF_KTP, 1, 5)
            pre_state()
            phi_feat(phiQ, OFF_QT, 0, 2)
            phi_feat(phiQ, OFF_QT, 1, 3)
            for j in range(2):
                a_quad(j)
            for j in range(4):
                g_pair(j)
            otiles = []
            for cp in range(4):
                Ot = ps_o.tile([C, 2 * VW], F32, tag="o", name=f"O{cp}")
                otiles.append(Ot)
            for c in range(NCH):
                o_chunk(c, otiles[c // 2])
            for (a, b, q) in cfg["out_pieces"]:
                dma_eng(q).dma_start(out=out[:, a:b], in_=outstage[:, a:b])

    nc.compile()
    return nc


def _get_nc(cfg=None):
    key = "nc" if cfg is None else repr(sorted((cfg or {}).items()))
    if key not in _cache:
        _cache[key] = _build(cfg)
    return _cache[key]


def _pack_inputs(q, k, v, W_phi, b_phi):
    import ml_dtypes
    bf16 = ml_dtypes.bfloat16

    WT = np.ascontiguousarray(W_phi.T)                    # [d, e]
    maskm = np.triu(np.ones((C, C), np.float32))          # keep tau <= t
    mask4 = np.concatenate([maskm] * 4, axis=1)           # [C, 512]
    ident = np.eye(C, dtype=np.float32)
    btile = np.tile(b_phi, 4).reshape(1, 512).astype(bf16)

    def aug(vh):  # [H, DV] -> [C, NCH*(DV+1)] partition-major with ones col
        a = np.concatenate([vh, np.ones((H, 1), np.float32)], axis=1)
        return a.reshape(NCH, C, VW).transpose(1, 0, 2).reshape(C, NCH * VW)

    zeros_vp = np.zeros((C, NCH * VW), np.float32)
    zeros_ktp = np.zeros((D, H), np.float32)

    in_maps = []
    for core in range(NCORES):
        b_idx, half = divmod(core, 2)
        sl = slice(half * H, (half + 1) * H)
        b16 = np.empty((D, B16_COLS), np.float32)
        b16[:, OFF_WTB:OFF_WTB + D] = WT
        b16[:, OFF_MASK4:OFF_MASK4 + 4 * C] = mask4
        b16[:, OFF_ID:OFF_ID + C] = ident
        b16[:, OFF_B] = b_phi
        b16[:, OFF_B1] = b_phi + 1.0
        b16[:, OFF_QT:OFF_QT + H] = q[b_idx, sl].T
        b16[:, OFF_KT:OFF_KT + H] = k[b_idx, sl].T
        if half == 1:
            b16[:, OFF_KTP:OFF_KTP + H] = k[b_idx, 0:H].T
            b16[:, OFF_VP:OFF_VP + NCH * VW] = aug(v[b_idx, 0:H])
        else:
            b16[:, OFF_KTP:OFF_KTP + H] = zeros_ktp
            b16[:, OFF_VP:OFF_VP + NCH * VW] = zeros_vp
        b16[:, OFF_V:OFF_V + NCH * VW] = aug(v[b_idx, sl])
        in_maps.append({"bin": b16.astype(bf16), "btile": btile})
    return in_maps


def kernel(q, k, v, W_phi, b_phi, cfg=None):
    from concourse.bass_utils import run_bass_kernel_spmd

    q = np.asarray(q, np.float32)
    k = np.asarray(k, np.float32)
    v = np.asarray(v, np.float32)
    W_phi = np.asarray(W_phi, np.float32)
    b_phi = np.asarray(b_phi, np.float32)

    in_maps = _pack_inputs(q, k, v, W_phi, b_phi)
    nc = _get_nc(cfg)
    res = run_bass_kernel_spmd(nc, in_maps, list(range(NCORES)))

    out = np.empty((B, T, DV), np.float32)
    for core in range(NCORES):
        b_idx, half = divmod(core, 2)
        o = np.asarray(res.results[core]["out"], dtype=np.float32)
        o = o.reshape(C, NCH, DV).transpose(1, 0, 2).reshape(H, DV)
        out[b_idx, half * H:(half + 1) * H] = o
    return out


# revision 10
# speedup vs baseline: 1.0626x; 1.0626x over previous
"""Causal linear attention (ELU+1 feature map) on 8 trn2 NeuronCores.

Sharding: core i handles batch b=i//2, sequence half h=i%2 (T=2048 -> 1024
tokens/core).  Second-half cores recompute the first half's running state
g_pre = sum_tau phi(k_tau) [v_tau, 1]  (128x129, col 128 = z) from k/v of the
first half; first-half cores get zeroed aux inputs so their g_pre == 0.

Key identity: phi(y) = ELU(y)+1 = min(exp(y), max(y+1, 1))
  -> per 512-col block: ACT exp, one tensor_scalar (add bias+1, max 1),
     one tensor_tensor min.

State handled as independent per-chunk-pair tiles g[j] (no serial snapshot
chain):  O_c = Am_c^T.T @ [V_c,1] + Q_c @ g_pre + sum_{j<c} Q_c @ g_j
  out_c = O_c[:, :128] * (1 / O_c[:, 128])

Token-major phi(K) for the state matmuls comes from DMA-engine transposes
(dma_start_transpose) of the feature-major phi(K), or PE transposes (cfg).
"""

import numpy as np

B, T, D, DV = 4, 2048, 128, 128
H = T // 2          # tokens per core
C = 128             # chunk
NCH = H // C        # chunks per half (8)
NCORES = 8
VW = DV + 1

# bf16 pack columns
OFF_WTB = 0
OFF_MASK4 = OFF_WTB + D          # 512-wide four-chunk causal mask
OFF_ID = OFF_MASK4 + 4 * C       # identity (PE transpose)
OFF_B = OFF_ID + C               # bias col
OFF_B1 = OFF_B + 1               # bias+1 col
OFF_KT = OFF_B1 + 1
OFF_KTP = OFF_KT + H
OFF_QT = OFF_KTP + H
OFF_VP = OFF_QT + H
OFF_V = OFF_VP + NCH * VW
B16_COLS = OFF_V + NCH * VW

CFG = {
    # DMA piece order: list of (colstart, colend, queue) — queue: sp|dve|act
    "pieces": [
        (OFF_WTB, OFF_KT + H, "sp"),      # consts + kT (contiguous)
        (OFF_KTP, OFF_KTP + H, "act"),    # kTp
        (OFF_VP, OFF_VP + NCH * VW, "pool"),  # vp
        (OFF_QT, OFF_QT + H, "sp"),       # qT
        (OFF_V, B16_COLS, "sp"),          # v
    ],
    # engines per phi block [K0, K1, Q0, Q1, P0, P1]
    "blk_style": ["fused", "fused", "fused", "fused", "reluact", "reluact"],
    "tt_eng": ["dve", "dve", "dve", "dve", "dve", "dve"],
    "ktok_mode": "pe",       # dmat | pe
    "ktok_copy": ["act", "dve"],   # per 4-chunk transpose batch
    "t_queue": "sync",
    "mask_eng": ["dve", "dve"],   # per quad
    "gcopy_eng": ["dve", "act", "dve", "act", "act"],  # 4 pairs + pre
    "scale_eng": ["dve", "act", "dve", "act", "dve", "act", "dve", "act"],
    "o_first": "av",          # first matmul in each O chain
    "out_pieces": [(0, 768, "sp"), (768, 1024, "sp")],
    "o_bufs": 2,
    "emit": "default",
}

_cache = {}


def _build(cfg=None):
    import concourse.bacc as bacc
    import concourse.tile as tile
    from concourse import mybir
    from bass_rust import add_dep_helper

    cfg = dict(CFG, **(cfg or {}))
    F32 = mybir.dt.float32
    BF16 = mybir.dt.bfloat16
    AF = mybir.ActivationFunctionType
    ALU = mybir.AluOpType

    nc = bacc.Bacc(None, target_bir_lowering=False, debug=False,
                   num_devices=NCORES)

    bin_ = nc.declare_dram_parameter("bin", [D, B16_COLS], BF16, isOutput=False)
    btile = nc.declare_dram_parameter("btile", [1, 512], BF16, isOutput=False)
    out = nc.declare_dram_parameter("out", [C, NCH * DV], BF16, isOutput=True)

    def dma_eng(which):
        return {"sp": nc.sync, "dve": nc.vector, "act": nc.scalar,
                "pool": nc.gpsimd}[which]

    with tile.TileContext(nc) as tc:
        with (
            tc.tile_pool(name="cst", bufs=1) as cst,
            tc.tile_pool(name="io", bufs=1) as io,
            tc.tile_pool(name="phi", bufs=1) as phip,
            tc.tile_pool(name="wrk", bufs=2) as wrk,
            tc.tile_pool(name="ps_phi", bufs=2, space="PSUM") as ps_phi,
            tc.tile_pool(name="ps_tr", bufs=1, space="PSUM") as ps_tr,
            tc.tile_pool(name="ps_ag", bufs=3, space="PSUM") as ps_ag,
            tc.tile_pool(name="ps_o", bufs=cfg["o_bufs"], space="PSUM") as ps_o,
        ):
            # ---- warm the ACT table while DMAs run ----
            s_warm = cst.tile([D, 1], F32, name="s_warm")
            nc.vector.memset(s_warm, 0.0)
            s_warm2 = cst.tile([D, 1], BF16, name="s_warm2")
            nc.scalar.activation(s_warm2, s_warm, AF.Exp)

            # ---- loads ----
            s_b16 = io.tile([D, B16_COLS], BF16, name="s_b16")
            s_btile = cst.tile([1, 512], BF16, name="s_btile")
            s_ones = cst.tile([1, C], BF16, name="s_ones")
            nc.vector.memset(s_ones, 1.0)
            for (a, b, q) in cfg["pieces"]:
                dma_eng(q).dma_start(out=s_b16[:, a:b], in_=bin_[:, a:b])
            nc.sync.dma_start(out=s_btile, in_=btile[:, :])

            s_b = s_b16[:, OFF_B:OFF_B + 1]
            s_b1_32 = cst.tile([D, 1], F32, name="s_b1_32")
            nc.vector.tensor_copy(s_b1_32, s_b16[:, OFF_B1:OFF_B1 + 1])
            sWTb = s_b16[:, OFF_WTB:OFF_WTB + D]
            s_mask4 = s_b16[:, OFF_MASK4:OFF_MASK4 + 4 * C]

            def vsl(c):
                return s_b16[:, OFF_V + VW * c:OFF_V + VW * (c + 1)]

            def vpsl(c):
                return s_b16[:, OFF_VP + VW * c:OFF_VP + VW * (c + 1)]

            phiK = phip.tile([D, H], BF16, name="phiK")
            phiQ = phip.tile([D, H], BF16, name="phiQ")
            phiT = phip.tile([C, H], BF16, name="phiT")   # token-major pre
            ktok = phip.tile([C, H], BF16, name="ktok")
            outstage = phip.tile([C, NCH * DV], BF16, name="outstage")

            # ---- feature-major phi block: dst[:, j*512:(j+1)*512] ----
            def phi_feat(dst, src_off, j, bi):
                pre = ps_phi.tile([D, 512], F32, tag="pp", name="pre")
                nc.tensor.matmul(pre, sWTb,
                                 s_b16[:, src_off + 512 * j:src_off + 512 * (j + 1)],
                                 start=True, stop=True)
                sl = slice(512 * j, 512 * (j + 1))
                e_t = wrk.tile([D, 512], BF16, tag="e", name="e_t")
                r_t = wrk.tile([D, 512], BF16, tag="r", name="r_t")
                nc.scalar.activation(e_t, pre, AF.Exp, bias=s_b, scale=1.0)
                if cfg["blk_style"][bi] == "fused":
                    # r1 = max(y+b+1, 1) on DVE from PSUM
                    nc.vector.tensor_scalar(out=r_t, in0=pre, scalar1=s_b1_32,
                                            scalar2=1.0, op0=ALU.add,
                                            op1=ALU.max)
                else:
                    # r = relu(y+b) on ACT, then r1 = r+1 on DVE (4x)
                    r0 = wrk.tile([D, 512], BF16, tag="r0", name="r0")
                    nc.scalar.activation(r0, pre, AF.Relu, bias=s_b, scale=1.0)
                    nc.vector.tensor_scalar(out=r_t, in0=r0, scalar1=1.0,
                                            scalar2=None, op0=ALU.add)
                nc.vector.tensor_tensor(out=dst[:, sl], in0=e_t, in1=r_t,
                                        op=ALU.min)

            # ---- token-major phi block (bias via ones-matmul) ----
            def phi_tok(dst, src_off, j, bi):
                pst = ps_phi.tile([C, 512], F32, tag="pp", name="pst")
                prev = nc.tensor.matmul(pst, s_ones, s_btile[:, :],
                                        start=True, stop=False)
                for cc in range(4):
                    mm = nc.tensor.matmul(
                        pst[:, C * cc:C * (cc + 1)],
                        s_b16[:, src_off + 512 * j + C * cc:
                              src_off + 512 * j + C * (cc + 1)],
                        sWTb, start=False, stop=(cc == 3))
                    add_dep_helper(mm.ins, prev.ins, sync=False,
                                   reason="psum group order")
                    prev = mm
                sl = slice(512 * j, 512 * (j + 1))
                e_t = wrk.tile([C, 512], BF16, tag="e", name="e_t")
                r_t = wrk.tile([C, 512], BF16, tag="r", name="r_t")
                nc.scalar.activation(e_t, pst, AF.Exp)
                if cfg["blk_style"][bi] == "fused":
                    nc.vector.tensor_scalar(out=r_t, in0=pst, scalar1=1.0,
                                            scalar2=1.0, op0=ALU.add,
                                            op1=ALU.max)
                else:
                    r0 = wrk.tile([C, 512], BF16, tag="r0", name="r0")
                    nc.scalar.activation(r0, pst, AF.Relu)
                    nc.vector.tensor_scalar(out=r_t, in0=r0, scalar1=1.0,
                                            scalar2=None, op0=ALU.add)
                nc.vector.tensor_tensor(out=dst[:, sl], in0=e_t, in1=r_t,
                                        op=ALU.min)

            # ---- ktok: token-major own-K ----
            def ktok_half(j):
                if cfg["ktok_mode"] == "dmat":
                    k3 = ktok[:, 512 * j:512 * (j + 1)].rearrange(
                        "p (c w) -> p c w", c=4)
                    dma_eng({"sync": "sp"}.get(cfg["t_queue"], cfg["t_queue"])
                            ).dma_start_transpose(k3, phiK[:, 512 * j:512 * (j + 1)])
                else:
                    trp = ps_tr.tile([C, 512], BF16, tag="tr", name="trp")
                    prev = None
                    for cc in range(4):
                        c = 4 * j + cc
                        mm = nc.tensor.transpose(trp[:, C * cc:C * (cc + 1)],
                                                 phiK[:, C * c:C * (c + 1)],
                                                 s_ident)
                        if prev is not None:
                            add_dep_helper(mm.ins, prev.ins, sync=False,
                                           reason="psum order")
                        prev = mm
                    eng = cfg["ktok_copy"][j]
                    sl = slice(512 * j, 512 * (j + 1))
                    if eng == "act":
                        nc.scalar.activation(ktok[:, sl], trp, AF.Copy)
                    elif eng == "pool":
                        nc.gpsimd.tensor_copy(ktok[:, sl], trp)
                    else:
                        nc.vector.tensor_copy(ktok[:, sl], trp)

            s_ident = s_b16[:, OFF_ID:OFF_ID + C]

            # ---- pre-state: 8 matmuls into one PSUM tile -> g_pre ----
            g_pre = phip.tile([D, VW], BF16, name="g_pre")

            def pre_state():
                S = ps_ag.tile([D, VW], F32, tag="ag", name="S")
                prev = None
                for c in range(NCH):
                    mm = nc.tensor.matmul(S, phiT[:, C * c:C * (c + 1)],
                                          vpsl(c), start=(c == 0),
                                          stop=(c == NCH - 1),
                                          skip_group_check=True)
                    if prev is not None:
                        add_dep_helper(mm.ins, prev.ins, sync=False,
                                       reason="psum group order")
                    prev = mm
                eng = cfg["gcopy_eng"][4]
                if eng == "act":
                    nc.scalar.activation(g_pre, S, AF.Copy)
                else:
                    nc.vector.tensor_copy(g_pre, S)

            # ---- G pairs ----
            g = [None] * 4

            def g_pair(j):
                Gp = ps_ag.tile([D, 2 * VW], F32, tag="ag", name="Gp")
                m0 = nc.tensor.matmul(Gp[:, 0:VW], ktok[:, C * 2 * j:C * (2 * j + 1)],
                                      vsl(2 * j), start=True, stop=True,
                                      skip_group_check=True)
                m1 = nc.tensor.matmul(Gp[:, VW:2 * VW],
                                      ktok[:, C * (2 * j + 1):C * (2 * j + 2)],
                                      vsl(2 * j + 1), start=True, stop=True,
                                      skip_group_check=True)
                add_dep_helper(m1.ins, m0.ins, sync=False, reason="psum order")
                gj = phip.tile([D, 2 * VW], BF16, name=f"g{j}")
                g[j] = gj
                eng = cfg["gcopy_eng"][j]
                if eng == "act":
                    nc.scalar.activation(gj, Gp, AF.Copy)
                else:
                    nc.vector.tensor_copy(gj, Gp)

            # ---- A quads + mask ----
            Am = [None] * 2

            def a_quad(j):
                Ap = ps_ag.tile([C, 4 * C], F32, tag="ag", name="Ap")
                prev = None
                for cc in range(4):
                    c = 4 * j + cc
                    mm = nc.tensor.matmul(Ap[:, C * cc:C * (cc + 1)],
                                          phiK[:, C * c:C * (c + 1)],
                                          phiQ[:, C * c:C * (c + 1)],
                                          start=True, stop=True,
                                          skip_group_check=True)
                    if prev is not None:
                        add_dep_helper(mm.ins, prev.ins, sync=False,
                                       reason="psum order")
                    prev = mm
                amj = phip.tile([C, 4 * C], BF16, name=f"am{j}")
                Am[j] = amj
                eng = cfg["mask_eng"][j]
                e = nc.vector if eng == "dve" else nc.gpsimd
                e.tensor_tensor(out=amj, in0=Ap, in1=s_mask4, op=ALU.mult)

            # ---- O chunks ----
            def o_chunk(c, Ot):
                half = c % 2
                osl = slice(half * VW, (half + 1) * VW)
                mms = []
                if cfg["o_first"] == "av":
                    mms.append(("av", None))
                    mms.append(("gpre", None))
                else:
                    mms.append(("gpre", None))
                    mms.append(("av", None))
                for j in range(c):
                    mms.append(("g", j))
                prev = None
                qsl = phiQ[:, C * c:C * (c + 1)]
                for i, (kind, j) in enumerate(mms):
                    start, stop = (i == 0), (i == len(mms) - 1)
                    if kind == "av":
                        amj = Am[c // 4]
                        mm = nc.tensor.matmul(
                            Ot[:, osl], amj[:, (c % 4) * C:(c % 4 + 1) * C],
                            vsl(c), start=start, stop=stop,
                            skip_group_check=True)
                    elif kind == "gpre":
                        mm = nc.tensor.matmul(Ot[:, osl], qsl, g_pre,
                                              start=start, stop=stop,
                                              skip_group_check=True)
                    else:
                        gj = g[j // 2]
                        gslice = gj[:, (j % 2) * VW:(j % 2 + 1) * VW]
                        mm = nc.tensor.matmul(Ot[:, osl], qsl, gslice,
                                              start=start, stop=stop,
                                              skip_group_check=True)
                    if prev is not None:
                        add_dep_helper(mm.ins, prev.ins, sync=False,
                                       reason="psum group order")
                    prev = mm
                # scale
                eng = cfg["scale_eng"][c]
                dsl = outstage[:, DV * c:DV * (c + 1)]
                ssl = Ot[:, half * VW:half * VW + DV]
                den = Ot[:, half * VW + DV:half * VW + DV + 1]
                if eng == "div":
                    nc.vector.tensor_scalar(out=dsl, in0=ssl, scalar1=den,
                                            scalar2=None, op0=ALU.divide)
                else:
                    rec = wrk.tile([C, 1], F32, tag="rec", name="rec")
                    nc.vector.reciprocal(rec, den)
                    if eng == "act":
                        nc.scalar.activation(dsl, ssl, AF.Copy, bias=0.0,
                                             scale=rec)
                    else:
                        nc.vector.tensor_scalar_mul(dsl, ssl, rec)

            # ================= emission =================
            phi_feat(phiK, OFF_KT, 0, 0)
            phi_feat(phiK, OFF_KT, 1, 1)
            ktok_half(0)
            phi_tok(phiT, OFF_KTP, 0, 4)
            phi_feat(phiQ, OFF_QT, 0, 2)
            ktok_half(1)
            phi_tok(phiT, OFF_KTP, 1, 5)
            phi_feat(phiQ, OFF_QT, 1, 3)
            pre_state()
            for j in range(2):
                a_quad(j)
            for j in range(4):
                g_pair(j)
            otiles = []
            for cp in range(4):
                Ot = ps_o.tile([C, 2 * VW], F32, tag="o", name=f"O{cp}")
                otiles.append(Ot)
            for c in range(NCH):
                o_chunk(c, otiles[c // 2])
            for (a, b, q) in cfg["out_pieces"]:
                dma_eng(q).dma_start(out=out[:, a:b], in_=outstage[:, a:b])

    nc.compile()
    return nc


def _get_nc(cfg=None):
    key = "nc" if cfg is None else repr(sorted((cfg or {}).items()))
    if key not in _cache:
        _cache[key] = _build(cfg)
    return _cache[key]


def _pack_inputs(q, k, v, W_phi, b_phi):
    import ml_dtypes
    bf16 = ml_dtypes.bfloat16

    WT = np.ascontiguousarray(W_phi.T)                    # [d, e]
    maskm = np.triu(np.ones((C, C), np.float32))          # keep tau <= t
    mask4 = np.concatenate([maskm] * 4, axis=1)           # [C, 512]
    ident = np.eye(C, dtype=np.float32)
    btile = np.tile(b_phi, 4).reshape(1, 512).astype(bf16)

    def aug(vh):  # [H, DV] -> [C, NCH*(DV+1)] partition-major with ones col
        a = np.concatenate([vh, np.ones((H, 1), np.float32)], axis=1)
        return a.reshape(NCH, C, VW).transpose(1, 0, 2).reshape(C, NCH * VW)

    zeros_vp = np.zeros((C, NCH * VW), np.float32)
    zeros_ktp = np.zeros((D, H), np.float32)

    in_maps = []
    for core in range(NCORES):
        b_idx, half = divmod(core, 2)
        sl = slice(half * H, (half + 1) * H)
        b16 = np.empty((D, B16_COLS), np.float32)
        b16[:, OFF_WTB:OFF_WTB + D] = WT
        b16[:, OFF_MASK4:OFF_MASK4 + 4 * C] = mask4
        b16[:, OFF_ID:OFF_ID + C] = ident
        b16[:, OFF_B] = b_phi
        b16[:, OFF_B1] = b_phi + 1.0
        b16[:, OFF_QT:OFF_QT + H] = q[b_idx, sl].T
        b16[:, OFF_KT:OFF_KT + H] = k[b_idx, sl].T
        if half == 1:
            b16[:, OFF_KTP:OFF_KTP + H] = k[b_idx, 0:H].T
            b16[:, OFF_VP:OFF_VP + NCH * VW] = aug(v[b_idx, 0:H])
        else:
            b16[:, OFF_KTP:OFF_KTP + H] = zeros_ktp
            b16[:, OFF_VP:OFF_VP + NCH * VW] = zeros_vp
        b16[:, OFF_V:OFF_V + NCH * VW] = aug(v[b_idx, sl])
        in_maps.append({"bin": b16.astype(bf16), "btile": btile})
    return in_maps


def kernel(q, k, v, W_phi, b_phi, cfg=None):
    from concourse.bass_utils import run_bass_kernel_spmd

    q = np.asarray(q, np.float32)
    k = np.asarray(k, np.float32)
    v = np.asarray(v, np.float32)
    W_phi = np.asarray(W_phi, np.float32)
    b_phi = np.asarray(b_phi, np.float32)

    in_maps = _pack_inputs(q, k, v, W_phi, b_phi)
    nc = _get_nc(cfg)
    res = run_bass_kernel_spmd(nc, in_maps, list(range(NCORES)))

    out = np.empty((B, T, DV), np.float32)
    for core in range(NCORES):
        b_idx, half = divmod(core, 2)
        o = np.asarray(res.results[core]["out"], dtype=np.float32)
        o = o.reshape(C, NCH, DV).transpose(1, 0, 2).reshape(H, DV)
        out[b_idx, half * H:(half + 1) * H] = o
    return out


# revision 11
# speedup vs baseline: 1.0982x; 1.0335x over previous
"""Causal linear attention (ELU+1 feature map) on 8 trn2 NeuronCores.

Sharding: core i handles batch b=i//2, sequence half h=i%2 (T=2048 -> 1024
tokens/core).  Second-half cores recompute the first half's running state
g_pre = sum_tau phi(k_tau) [v_tau, 1]  (128x129, col 128 = z) from k/v of the
first half; first-half cores get zeroed aux inputs so their g_pre == 0.

Key identity: phi(y) = ELU(y)+1 = min(exp(y), max(y+1, 1))
  -> per 512-col block: ACT exp, one tensor_scalar (add bias+1, max 1),
     one tensor_tensor min.

State handled as independent per-chunk-pair tiles g[j] (no serial snapshot
chain):  O_c = Am_c^T.T @ [V_c,1] + Q_c @ g_pre + sum_{j<c} Q_c @ g_j
  out_c = O_c[:, :128] * (1 / O_c[:, 128])

Token-major phi(K) for the state matmuls comes from DMA-engine transposes
(dma_start_transpose) of the feature-major phi(K), or PE transposes (cfg).
"""

import numpy as np

B, T, D, DV = 4, 2048, 128, 128
H = T // 2          # tokens per core
C = 128             # chunk
NCH = H // C        # chunks per half (8)
NCORES = 8
VW = DV + 1

# bf16 pack columns
OFF_WTB = 0
OFF_MASK4 = OFF_WTB + D          # 512-wide four-chunk causal mask
OFF_ID = OFF_MASK4 + 4 * C       # identity (PE transpose)
OFF_B = OFF_ID + C               # bias col
OFF_B1 = OFF_B + 1               # bias+1 col
OFF_KT = OFF_B1 + 1
OFF_KTP = OFF_KT + H
OFF_QT = OFF_KTP + H
OFF_VP = OFF_QT + H
OFF_V = OFF_VP + NCH * VW
B16_COLS = OFF_V + NCH * VW

CFG = {
    # DMA piece order: list of (colstart, colend, queue) — queue: sp|dve|act
    "pieces": [
        (OFF_WTB, OFF_KT + H, "sp"),      # consts + kT (contiguous)
        (OFF_KTP, OFF_KTP + H, "act"),    # kTp
        (OFF_VP, OFF_VP + NCH * VW, "pool"),  # vp
        (OFF_QT, OFF_QT + H, "sp"),       # qT
        (OFF_V, B16_COLS, "sp"),          # v
    ],
    # engines per phi block [K0, K1, Q0, Q1, P0, P1]
    "blk_style": ["fused", "fused", "fused", "fused", "reluact", "reluact"],
    "tt_eng": ["dve", "dve", "dve", "dve", "dve", "dve"],
    "ktok_mode": "dmat",     # dmat | pe
    "ktok_copy": ["act", "dve"],   # per 4-chunk transpose batch
    "t_queue": "sync",
    "mask_eng": ["dve", "dve"],   # per quad
    "gcopy_eng": ["dve", "act", "dve", "act", "act"],  # 4 pairs + pre
    "scale_eng": ["dve", "act", "dve", "act", "dve", "act", "dve", "act"],
    "o_first": "av",          # first matmul in each O chain
    "out_pieces": [(0, 768, "sp"), (768, 1024, "sp")],
    "o_bufs": 3,
    "emit": "default",
}

_cache = {}


def _build(cfg=None):
    import concourse.bacc as bacc
    import concourse.tile as tile
    from concourse import mybir
    from bass_rust import add_dep_helper

    cfg = dict(CFG, **(cfg or {}))
    F32 = mybir.dt.float32
    BF16 = mybir.dt.bfloat16
    AF = mybir.ActivationFunctionType
    ALU = mybir.AluOpType

    nc = bacc.Bacc(None, target_bir_lowering=False, debug=False,
                   num_devices=NCORES)

    bin_ = nc.declare_dram_parameter("bin", [D, B16_COLS], BF16, isOutput=False)
    btile = nc.declare_dram_parameter("btile", [1, 512], BF16, isOutput=False)
    out = nc.declare_dram_parameter("out", [C, NCH * DV], BF16, isOutput=True)

    def dma_eng(which):
        return {"sp": nc.sync, "dve": nc.vector, "act": nc.scalar,
                "pool": nc.gpsimd}[which]

    with tile.TileContext(nc) as tc:
        with (
            tc.tile_pool(name="cst", bufs=1) as cst,
            tc.tile_pool(name="io", bufs=1) as io,
            tc.tile_pool(name="phi", bufs=1) as phip,
            tc.tile_pool(name="wrk", bufs=2) as wrk,
            tc.tile_pool(name="ps_phi", bufs=2, space="PSUM") as ps_phi,
            tc.tile_pool(name="ps_tr", bufs=1, space="PSUM") as ps_tr,
            tc.tile_pool(name="ps_ag", bufs=3, space="PSUM") as ps_ag,
            tc.tile_pool(name="ps_o", bufs=cfg["o_bufs"], space="PSUM") as ps_o,
        ):
            # ---- warm the ACT table while DMAs run ----
            s_warm = cst.tile([D, 1], F32, name="s_warm")
            nc.vector.memset(s_warm, 0.0)
            s_warm2 = cst.tile([D, 1], BF16, name="s_warm2")
            nc.scalar.activation(s_warm2, s_warm, AF.Exp)

            # ---- loads ----
            s_b16 = io.tile([D, B16_COLS], BF16, name="s_b16")
            s_btile = cst.tile([1, 512], BF16, name="s_btile")
            s_ones = cst.tile([1, C], BF16, name="s_ones")
            nc.vector.memset(s_ones, 1.0)
            for idx, (a, b, q) in enumerate(cfg["pieces"]):
                dma_eng(q).dma_start(out=s_b16[:, a:b], in_=bin_[:, a:b])
                if idx == 0:
                    nc.sync.dma_start(out=s_btile, in_=btile[:, :])

            s_b = s_b16[:, OFF_B:OFF_B + 1]
            s_b1_32 = cst.tile([D, 1], F32, name="s_b1_32")
            nc.vector.tensor_copy(s_b1_32, s_b16[:, OFF_B1:OFF_B1 + 1])
            sWTb = s_b16[:, OFF_WTB:OFF_WTB + D]
            s_mask4 = s_b16[:, OFF_MASK4:OFF_MASK4 + 4 * C]

            def vsl(c):
                return s_b16[:, OFF_V + VW * c:OFF_V + VW * (c + 1)]

            def vpsl(c):
                return s_b16[:, OFF_VP + VW * c:OFF_VP + VW * (c + 1)]

            phiK = phip.tile([D, H], BF16, name="phiK")
            phiQ = phip.tile([D, H], BF16, name="phiQ")
            phiT = phip.tile([C, H], BF16, name="phiT")   # token-major pre
            ktok = phip.tile([C, H], BF16, name="ktok")
            outstage = phip.tile([C, NCH * DV], BF16, name="outstage")

            # ---- feature-major phi block: dst[:, j*512:(j+1)*512] ----
            def phi_feat(dst, src_off, j, bi):
                pre = ps_phi.tile([D, 512], F32, tag="pp", name="pre")
                nc.tensor.matmul(pre, sWTb,
                                 s_b16[:, src_off + 512 * j:src_off + 512 * (j + 1)],
                                 start=True, stop=True)
                sl = slice(512 * j, 512 * (j + 1))
                e_t = wrk.tile([D, 512], BF16, tag="e", name="e_t")
                r_t = wrk.tile([D, 512], BF16, tag="r", name="r_t")
                nc.scalar.activation(e_t, pre, AF.Exp, bias=s_b, scale=1.0)
                if cfg["blk_style"][bi] == "fused":
                    # r1 = max(y+b+1, 1) on DVE from PSUM
                    nc.vector.tensor_scalar(out=r_t, in0=pre, scalar1=s_b1_32,
                                            scalar2=1.0, op0=ALU.add,
                                            op1=ALU.max)
                else:
                    # r = relu(y+b) on ACT, then r1 = r+1 on DVE (4x)
                    r0 = wrk.tile([D, 512], BF16, tag="r0", name="r0")
                    nc.scalar.activation(r0, pre, AF.Relu, bias=s_b, scale=1.0)
                    nc.vector.tensor_scalar(out=r_t, in0=r0, scalar1=1.0,
                                            scalar2=None, op0=ALU.add)
                nc.vector.tensor_tensor(out=dst[:, sl], in0=e_t, in1=r_t,
                                        op=ALU.min)

            # ---- token-major phi block (bias via ones-matmul) ----
            def phi_tok(dst, src_off, j, bi):
                pst = ps_phi.tile([C, 512], F32, tag="pp", name="pst")
                prev = nc.tensor.matmul(pst, s_ones, s_btile[:, :],
                                        start=True, stop=False)
                for cc in range(4):
                    mm = nc.tensor.matmul(
                        pst[:, C * cc:C * (cc + 1)],
                        s_b16[:, src_off + 512 * j + C * cc:
                              src_off + 512 * j + C * (cc + 1)],
                        sWTb, start=False, stop=(cc == 3))
                    add_dep_helper(mm.ins, prev.ins, sync=False,
                                   reason="psum group order")
                    prev = mm
                sl = slice(512 * j, 512 * (j + 1))
                e_t = wrk.tile([C, 512], BF16, tag="e", name="e_t")
                r_t = wrk.tile([C, 512], BF16, tag="r", name="r_t")
                nc.scalar.activation(e_t, pst, AF.Exp)
                if cfg["blk_style"][bi] == "fused":
                    nc.vector.tensor_scalar(out=r_t, in0=pst, scalar1=1.0,
                                            scalar2=1.0, op0=ALU.add,
                                            op1=ALU.max)
                else:
                    r0 = wrk.tile([C, 512], BF16, tag="r0", name="r0")
                    nc.scalar.activation(r0, pst, AF.Relu)
                    nc.vector.tensor_scalar(out=r_t, in0=r0, scalar1=1.0,
                                            scalar2=None, op0=ALU.add)
                nc.vector.tensor_tensor(out=dst[:, sl], in0=e_t, in1=r_t,
                                        op=ALU.min)

            # ---- ktok: token-major own-K ----
            def ktok_half(j):
                if cfg["ktok_mode"] == "dmat":
                    k3 = ktok[:, 512 * j:512 * (j + 1)].rearrange(
                        "p (c w) -> p c w", c=4)
                    dma_eng({"sync": "sp"}.get(cfg["t_queue"], cfg["t_queue"])
                            ).dma_start_transpose(k3, phiK[:, 512 * j:512 * (j + 1)])
                else:
                    trp = ps_tr.tile([C, 512], BF16, tag="tr", name="trp")
                    prev = None
                    for cc in range(4):
                        c = 4 * j + cc
                        mm = nc.tensor.transpose(trp[:, C * cc:C * (cc + 1)],
                                                 phiK[:, C * c:C * (c + 1)],
                                                 s_ident)
                        if prev is not None:
                            add_dep_helper(mm.ins, prev.ins, sync=False,
                                           reason="psum order")
                        prev = mm
                    eng = cfg["ktok_copy"][j]
                    sl = slice(512 * j, 512 * (j + 1))
                    if eng == "act":
                        nc.scalar.activation(ktok[:, sl], trp, AF.Copy)
                    elif eng == "pool":
                        nc.gpsimd.tensor_copy(ktok[:, sl], trp)
                    else:
                        nc.vector.tensor_copy(ktok[:, sl], trp)

            s_ident = s_b16[:, OFF_ID:OFF_ID + C]

            # ---- pre-state: 8 matmuls into one PSUM tile -> g_pre ----
            g_pre = phip.tile([D, VW], BF16, name="g_pre")

            def pre_state():
                S = ps_ag.tile([D, VW], F32, tag="ag", name="S")
                prev = None
                for c in range(NCH):
                    mm = nc.tensor.matmul(S, phiT[:, C * c:C * (c + 1)],
                                          vpsl(c), start=(c == 0),
                                          stop=(c == NCH - 1),
                                          skip_group_check=True)
                    if prev is not None:
                        add_dep_helper(mm.ins, prev.ins, sync=False,
                                       reason="psum group order")
                    prev = mm
                eng = cfg["gcopy_eng"][4]
                if eng == "act":
                    nc.scalar.activation(g_pre, S, AF.Copy)
                else:
                    nc.vector.tensor_copy(g_pre, S)

            # ---- G pairs ----
            g = [None] * 4

            def g_pair(j):
                Gp = ps_ag.tile([D, 2 * VW], F32, tag="ag", name="Gp")
                m0 = nc.tensor.matmul(Gp[:, 0:VW], ktok[:, C * 2 * j:C * (2 * j + 1)],
                                      vsl(2 * j), start=True, stop=True,
                                      skip_group_check=True)
                m1 = nc.tensor.matmul(Gp[:, VW:2 * VW],
                                      ktok[:, C * (2 * j + 1):C * (2 * j + 2)],
                                      vsl(2 * j + 1), start=True, stop=True,
                                      skip_group_check=True)
                add_dep_helper(m1.ins, m0.ins, sync=False, reason="psum order")
                gj = phip.tile([D, 2 * VW], BF16, name=f"g{j}")
                g[j] = gj
                eng = cfg["gcopy_eng"][j]
                if eng == "act":
                    nc.scalar.activation(gj, Gp, AF.Copy)
                else:
                    nc.vector.tensor_copy(gj, Gp)

            # ---- A quads + mask ----
            Am = [None] * 2

            def a_quad(j):
                Ap = ps_ag.tile([C, 4 * C], F32, tag="ag", name="Ap")
                prev = None
                for cc in range(4):
                    c = 4 * j + cc
                    mm = nc.tensor.matmul(Ap[:, C * cc:C * (cc + 1)],
                                          phiK[:, C * c:C * (c + 1)],
                                          phiQ[:, C * c:C * (c + 1)],
                                          start=True, stop=True,
                                          skip_group_check=True)
                    if prev is not None:
                        add_dep_helper(mm.ins, prev.ins, sync=False,
                                       reason="psum order")
                    prev = mm
                amj = phip.tile([C, 4 * C], BF16, name=f"am{j}")
                Am[j] = amj
                eng = cfg["mask_eng"][j]
                e = nc.vector if eng == "dve" else nc.gpsimd
                e.tensor_tensor(out=amj, in0=Ap, in1=s_mask4, op=ALU.mult)

            # ---- O chunks ----
            def o_chunk(c, Ot):
                half = c % 2
                osl = slice(half * VW, (half + 1) * VW)
                mms = []
                if cfg["o_first"] == "av":
                    mms.append(("av", None))
                    mms.append(("gpre", None))
                else:
                    mms.append(("gpre", None))
                    mms.append(("av", None))
                for j in range(c):
                    mms.append(("g", j))
                prev = None
                qsl = phiQ[:, C * c:C * (c + 1)]
                for i, (kind, j) in enumerate(mms):
                    start, stop = (i == 0), (i == len(mms) - 1)
                    if kind == "av":
                        amj = Am[c // 4]
                        mm = nc.tensor.matmul(
                            Ot[:, osl], amj[:, (c % 4) * C:(c % 4 + 1) * C],
                            vsl(c), start=start, stop=stop,
                            skip_group_check=True)
                    elif kind == "gpre":
                        mm = nc.tensor.matmul(Ot[:, osl], qsl, g_pre,
                                              start=start, stop=stop,
                                              skip_group_check=True)
                    else:
                        gj = g[j // 2]
                        gslice = gj[:, (j % 2) * VW:(j % 2 + 1) * VW]
                        mm = nc.tensor.matmul(Ot[:, osl], qsl, gslice,
                                              start=start, stop=stop,
                                              skip_group_check=True)
                    if prev is not None:
                        add_dep_helper(mm.ins, prev.ins, sync=False,
                                       reason="psum group order")
                    prev = mm
                # scale
                eng = cfg["scale_eng"][c]
                dsl = outstage[:, DV * c:DV * (c + 1)]
                ssl = Ot[:, half * VW:half * VW + DV]
                den = Ot[:, half * VW + DV:half * VW + DV + 1]
                if eng == "div":
                    nc.vector.tensor_scalar(out=dsl, in0=ssl, scalar1=den,
                                            scalar2=None, op0=ALU.divide)
                else:
                    rec = wrk.tile([C, 1], F32, tag="rec", name="rec")
                    nc.vector.reciprocal(rec, den)
                    if eng == "act":
                        nc.scalar.activation(dsl, ssl, AF.Copy, bias=0.0,
                                             scale=rec)
                    else:
                        nc.vector.tensor_scalar_mul(dsl, ssl, rec)

            # ================= emission =================
            phi_feat(phiK, OFF_KT, 0, 0)
            phi_feat(phiK, OFF_KT, 1, 1)
            ktok_half(0)
            phi_tok(phiT, OFF_KTP, 0, 4)
            phi_feat(phiQ, OFF_QT, 0, 2)
            ktok_half(1)
            phi_tok(phiT, OFF_KTP, 1, 5)
            phi_feat(phiQ, OFF_QT, 1, 3)
            pre_state()
            for j in range(2):
                a_quad(j)
            for j in range(4):
                g_pair(j)
            otiles = []
            for cp in range(4):
                Ot = ps_o.tile([C, 2 * VW], F32, tag="o", name=f"O{cp}")
                otiles.append(Ot)
            for c in range(NCH):
                o_chunk(c, otiles[c // 2])
            for (a, b, q) in cfg["out_pieces"]:
                dma_eng(q).dma_start(out=out[:, a:b], in_=outstage[:, a:b])

    nc.compile()
    return nc


def _get_nc(cfg=None):
    key = "nc" if cfg is None else repr(sorted((cfg or {}).items()))
    if key not in _cache:
        _cache[key] = _build(cfg)
    return _cache[key]


def _pack_inputs(q, k, v, W_phi, b_phi):
    import ml_dtypes
    bf16 = ml_dtypes.bfloat16

    WT = np.ascontiguousarray(W_phi.T)                    # [d, e]
    maskm = np.triu(np.ones((C, C), np.float32))          # keep tau <= t
    mask4 = np.concatenate([maskm] * 4, axis=1)           # [C, 512]
    ident = np.eye(C, dtype=np.float32)
    btile = np.tile(b_phi, 4).reshape(1, 512).astype(bf16)

    def aug(vh):  # [H, DV] -> [C, NCH*(DV+1)] partition-major with ones col
        a = np.concatenate([vh, np.ones((H, 1), np.float32)], axis=1)
        return a.reshape(NCH, C, VW).transpose(1, 0, 2).reshape(C, NCH * VW)

    zeros_vp = np.zeros((C, NCH * VW), np.float32)
    zeros_ktp = np.zeros((D, H), np.float32)

    in_maps = []
    for core in range(NCORES):
        b_idx, half = divmod(core, 2)
        sl = slice(half * H, (half + 1) * H)
        b16 = np.empty((D, B16_COLS), np.float32)
        b16[:, OFF_WTB:OFF_WTB + D] = WT
        b16[:, OFF_MASK4:OFF_MASK4 + 4 * C] = mask4
        b16[:, OFF_ID:OFF_ID + C] = ident
        b16[:, OFF_B] = b_phi
        b16[:, OFF_B1] = b_phi + 1.0
        b16[:, OFF_QT:OFF_QT + H] = q[b_idx, sl].T
        b16[:, OFF_KT:OFF_KT + H] = k[b_idx, sl].T
        if half == 1:
            b16[:, OFF_KTP:OFF_KTP + H] = k[b_idx, 0:H].T
            b16[:, OFF_VP:OFF_VP + NCH * VW] = aug(v[b_idx, 0:H])
        else:
            b16[:, OFF_KTP:OFF_KTP + H] = zeros_ktp
            b16[:, OFF_VP:OFF_VP + NCH * VW] = zeros_vp
        b16[:, OFF_V:OFF_V + NCH * VW] = aug(v[b_idx, sl])
        in_maps.append({"bin": b16.astype(bf16), "btile": btile})
    return in_maps


def kernel(q, k, v, W_phi, b_phi, cfg=None):
    from concourse.bass_utils import run_bass_kernel_spmd

    q = np.asarray(q, np.float32)
    k = np.asarray(k, np.float32)
    v = np.asarray(v, np.float32)
    W_phi = np.asarray(W_phi, np.float32)
    b_phi = np.asarray(b_phi, np.float32)

    in_maps = _pack_inputs(q, k, v, W_phi, b_phi)
    nc = _get_nc(cfg)
    res = run_bass_kernel_spmd(nc, in_maps, list(range(NCORES)))

    out = np.empty((B, T, DV), np.float32)
    for core in range(NCORES):
        b_idx, half = divmod(core, 2)
        o = np.asarray(res.results[core]["out"], dtype=np.float32)
        o = o.reshape(C, NCH, DV).transpose(1, 0, 2).reshape(H, DV)
        out[b_idx, half * H:(half + 1) * H] = o
    return out
